# revision 6
# baseline (speedup 1.0000x reference)
"""BERT self-attention (B=16, T=512, C=768, H=12, D=64) on 8 trn2 NeuronCores.

Data-parallel over batch: each core gets 2 batches. Matmul operands are fp16
(11-bit mantissa, ~tf32-class precision, 1 cycle/row PE streaming, FWL weight
loads); all accumulation stays fp32 in PSUM. Per core:
  xT    = x transposed during load via the DMA XBAR transpose (fp16).
  Q^T/K^T ([feature, token] layout, lhsT = W_attn tile) and V ([token, feature]
          layout with an interleaved ones column per head, lhsT = xT tile).
  S^T   = K^T-as-lhsT matmul -> scores in [key, query] layout (K=64, head pairs
          packed in PE row groups via base-partition-64 slices).
  P     = exp(S/8 + mask) on ScalarE (mask is a per-partition bias in this
          layout), written as fp16.
  y^T   = lhsT=[V_h | ones] matmul -> unnormalized y^T plus softmax row-sums as
          an extra PSUM row; each head's row-sum is inverted in place with a
          single-pass approx reciprocal, broadcast across 64 partitions by a
          stride-0 DMA, and applied with a DVE multiply fused with the PSUM
          drain.
  out   = y^T-as-lhsT matmul with W_proj + b_proj (fp32 result to DRAM).
Biases are folded in as K=1 matmuls against a ones row / fused into drains.
"""

import sys

sys.path.insert(0, "/opt/trn_rl_repo")

from contextlib import ExitStack

import numpy as np

B, T, C = 16, 512, 768
H, D = 12, 64
C3 = 3 * C
N_CORES = 8
BC = B // N_CORES           # batches per core
M = BC * T                  # tokens per core
KT = C // 128               # feature k-tiles (6)
TT = M // 128               # token tiles per core (8)
NQK = 2 * C // 128          # q+k feature n-tiles (12)
VW = H * 128                # v tile width: per-head [V_h | ones] blocks
SCALE = 1.0 / np.sqrt(D)

_cache = {}


def _build():
    import concourse.bass as bass
    import concourse.tile as tile
    from concourse import bacc, mybir
    f32 = mybir.dt.float32
    f16 = mybir.dt.float16
    Exp = mybir.ActivationFunctionType.Exp
    Mult = mybir.AluOpType.mult
    Add = mybir.AluOpType.add

    nc = bacc.Bacc("TRN2", target_bir_lowering=False, debug=False,
                   num_devices=N_CORES)
    x_d = nc.dram_tensor("x", [M, C], f16, kind="ExternalInput").ap()
    mask_d = nc.dram_tensor("mask", [BC, T], f32, kind="ExternalInput").ap()
    wa_d = nc.dram_tensor("w_attn", [C, C3], f16, kind="ExternalInput").ap()
    ba_d = nc.dram_tensor("b_attn", [1, C3], f16, kind="ExternalInput").ap()
    wp_d = nc.dram_tensor("w_proj", [C, C], f16, kind="ExternalInput").ap()
    bp_d = nc.dram_tensor("b_proj", [1, C], f16, kind="ExternalInput").ap()
    out_d = nc.dram_tensor("out", [M, C], f32, kind="ExternalOutput").ap()

    with tile.TileContext(nc) as tc, ExitStack() as ctx:
        pp = ctx.enter_context(tc.tile_pool(name="pp", bufs=1))
        np_ = ctx.enter_context(tc.tile_pool(name="norm", bufs=4))
        ap_ = ctx.enter_context(tc.tile_pool(name="att", bufs=6))
        ps_mm = ctx.enter_context(tc.tile_pool(name="ps_mm", bufs=2, space="PSUM"))

        # --- persistent SBUF tensors ---
        ones = pp.tile([1, M], f16, tag="ones")
        nc.vector.memset(ones[:], 1.0)
        mask_sb = pp.tile([128, BC * 4], f32, tag="mask")
        ba_t = pp.tile([1, C3], f16, tag="ba")
        bp_t = pp.tile([1, C], f16, tag="bp")
        ba_qk = pp.tile([128, NQK], f32, tag="ba_qk")
        ba_v_rep = pp.tile([128, C], f32, tag="ba_v_rep")
        bp_rep = pp.tile([128, C], f32, tag="bp_rep")

        xT = [pp.tile([128, M], f16, tag=f"xT{k}", name=f"xT{k}")
              for k in range(KT)]
        wa_qk = pp.tile([128, KT * 2 * C], f16, tag="wa_qk")   # [p, (k, n)]
        wa_v = pp.tile([128, KT * C], f16, tag="wa_v")         # [p, (k, n)]
        wp_t = pp.tile([128, KT * C], f16, tag="wp")           # [p, (k, n)]
        qkT = [pp.tile([128, M], f16, tag=f"qk{n}", name=f"qk{n}")
               for n in range(NQK)]
        v_t = [pp.tile([128, VW], f16, tag=f"v{t}", name=f"v{t}")
               for t in range(TT)]
        yT_t = [pp.tile([128, M], f16, tag=f"yT{c}", name=f"yT{c}")
                for c in range(KT)]

        # PE warm-up: harmless K=1 matmuls so the HAM un-throttles before the
        # real chains begin (outputs overwritten by start=True later).
        for w in range(16):
            pw = ps_mm.tile([128, 512], f32, tag="mm", name=f"warm{w}")
            nc.tensor.matmul(pw[:, 0:256], ones[0:1, 0:128],
                             ones[0:1, 0:256], start=True, stop=True)

        # --- input DMAs, consumption order, both HWDGE rings ---
        # sync ring: xbar-transpose loads of x (b0 tiles first);
        # scalar ring: weights interleaved with the odd-k transposes.
        def xtr(k, b):
            q = nc.sync if k % 2 == 0 else nc.scalar
            q.dma_start_transpose(
                xT[k][:, b * T:(b + 1) * T],
                x_d[b * T:(b + 1) * T, k * 128:(k + 1) * 128])

        wa_src = wa_d.rearrange("(k p) n -> p k n", p=128)
        wa_qk3 = wa_qk.rearrange("p (k n) -> p k n", n=2 * C)
        for k in (0, 2, 4):
            xtr(k, 0)
        xtr(1, 0)
        nc.scalar.dma_start(wa_qk3[:, :, 0:C], wa_src[:, :, 0:C])
        xtr(3, 0)
        xtr(5, 0)
        nc.scalar.dma_start(wa_qk3[:, :, C:2 * C], wa_src[:, :, C:2 * C])
        nc.scalar.dma_start(
            wa_v.rearrange("p (k n) -> p k n", n=C), wa_src[:, :, 2 * C:C3])
        for k in range(KT):
            xtr(k, 1)
        nc.scalar.dma_start(
            wp_t.rearrange("p (k n) -> p k n", n=C),
            wp_d.rearrange("(k p) n -> p k n", p=128))
        # small tables via SWDGE
        nc.gpsimd.dma_start(
            mask_sb[:],
            mask_d.rearrange("a b -> (a b)").rearrange("(j p) -> p j", p=128))
        nc.gpsimd.dma_start(ba_t[:], ba_d[:])
        nc.gpsimd.dma_start(bp_t[:], bp_d[:])
        nc.gpsimd.dma_start(
            ba_qk[:],
            ba_d[0, 0:2 * C].rearrange("(j p) -> p j", p=128))

        # replicate V/proj biases across partitions (K=1 matmuls)
        for lo, w in ((0, 512), (512, 256)):
            p = ps_mm.tile([128, 512], f32, tag="mm", name=f"brep{lo}")
            nc.tensor.matmul(
                p[:, :w], ones[0:1, 0:128],
                ba_t[0:1, 2 * C + lo:2 * C + lo + w], start=True, stop=True)
            nc.vector.tensor_copy(ba_v_rep[:, lo:lo + w], p[:, :w])
            p2 = ps_mm.tile([128, 512], f32, tag="mm", name=f"bprep{lo}")
            nc.tensor.matmul(
                p2[:, :w], ones[0:1, 0:128],
                bp_t[0:1, lo:lo + w], start=True, stop=True)
            nc.vector.tensor_copy(bp_rep[:, lo:lo + w], p2[:, :w])

        # ones columns inside the per-head V blocks (col D of each 128 block)
        for t in range(TT):
            nc.vector.memset(
                v_t[t].rearrange("p (h c) -> p h c", c=128)
                    [:, :, D:D + 1], 1.0)

        ps_s = ctx.enter_context(tc.tile_pool(name="ps_s", bufs=2, space="PSUM"))
        ps_y = ctx.enter_context(tc.tile_pool(name="ps_y", bufs=2, space="PSUM"))

        Ident = mybir.ActivationFunctionType.Identity

        def qkv_chain(b, i):
            """i in [0, 20): 12 QK n-tiles then 8 V half-tiles."""
            bcol = b * T
            if i < NQK:
                n = i
                p = ps_mm.tile([128, 512], f32, tag="mm", name=f"mm{b}_{i}")
                for k in range(KT):
                    nc.tensor.matmul(
                        p[:],
                        wa_qk[:, k * 2 * C + n * 128:k * 2 * C + (n + 1) * 128],
                        xT[k][:, bcol:bcol + T],
                        start=(k == 0), stop=(k == KT - 1))
                # drain+bias on ScalarE (idle early / has slack) to keep the
                # DVE from gating ps_mm reuse; half stay on DVE for b1.
                if b == 0 or i % 2 == 0:
                    nc.scalar.activation(
                        qkT[n][:, bcol:bcol + T], p[:], Ident,
                        bias=ba_qk[:, n:n + 1])
                else:
                    nc.vector.tensor_scalar_add(
                        qkT[n][:, bcol:bcol + T], p[:], ba_qk[:, n:n + 1])
            else:
                j = i - NQK
                t = b * 4 + j // 2
                lo, w = ((0, 512), (512, 256))[j % 2]
                p = ps_mm.tile([128, 512], f32, tag="mm", name=f"mm{b}_{i}")
                for k in range(KT):
                    nc.tensor.matmul(
                        p[:, :w],
                        xT[k][:, t * 128:(t + 1) * 128],
                        wa_v[:, k * C + lo:k * C + lo + w],
                        start=(k == 0), stop=(k == KT - 1))
                h0 = lo // D
                nc.vector.tensor_tensor(
                    out=v_t[t].rearrange("p (h c) -> p h c", c=128)
                        [:, h0:h0 + w // D, 0:D],
                    in0=p[:, :w].rearrange("p (h c) -> p h c", c=D),
                    in1=ba_v_rep[:, lo:lo + w].rearrange(
                        "p (h c) -> p h c", c=D),
                    op=Add)

        def attention_hp(b, hp):
            bcol = b * T
            e_tiles = []
            for kt in range(4):
                ps = ps_s.tile([128, 1024], f32)
                for sub in range(2):
                    r0 = 64 * sub
                    nc.tensor.matmul(
                        ps[:, sub * 512:sub * 512 + 512],
                        qkT[6 + hp][r0:r0 + D,
                                    bcol + kt * 128:bcol + (kt + 1) * 128],
                        qkT[hp][r0:r0 + D, bcol:bcol + T],
                        start=True, stop=True)
                e = ap_.tile([128, 1024], f16, tag="e")
                nc.scalar.activation(
                    e[:], ps[:], Exp,
                    bias=mask_sb[:, b * 4 + kt:b * 4 + kt + 1],
                    scale=float(SCALE))
                e_tiles.append(e)
            for sub in range(2):
                h = 2 * hp + sub
                py = ps_y.tile([128, 512], f32)
                for kt in range(4):
                    nc.tensor.matmul(
                        py[0:D + 1, :],
                        v_t[b * 4 + kt][:, 128 * h:128 * h + D + 1],
                        e_tiles[kt][:, sub * 512:sub * 512 + 512],
                        start=(kt == 0), stop=(kt == 3))
                # per-head softmax denominators: 1/rowsum straight off PSUM
                recip = np_.tile([D + 1, 512], f32, tag="recip", bufs=3)
                nc.vector.reciprocal_approx_fast(
                    recip[D:D + 1, :], py[D:D + 1, :])
                rep = np_.tile([D, 512], f32, tag="rep", bufs=3)
                nc.gpsimd.partition_broadcast(rep[:], recip[D:D + 1, :])
                nt, r0 = h // 2, 64 * (h % 2)
                dst = yT_t[nt][r0:r0 + D, bcol:bcol + T]
                if r0 == 0:
                    nc.vector.tensor_tensor(
                        out=dst, in0=py[0:D, :], in1=rep[:], op=Mult)
                else:
                    st = np_.tile([D, 512], f16, tag="st", bufs=3)
                    nc.vector.tensor_tensor(
                        out=st[:], in0=py[0:D, :], in1=rep[:], op=Mult)
                    nc.sync.dma_start(dst, st[:])

        pj_part = {}

        def proj_chunk(b, i, ks=0, ke=KT, partial=False):
            t = b * 4 + i // 2
            lo, w = ((0, 512), (512, 256))[i % 2]
            p = ps_mm.tile([128, 512], f32, tag="mm", name=f"pj{b}_{i}_{ks}")
            for k in range(ks, ke):
                nc.tensor.matmul(
                    p[:, :w],
                    yT_t[k][:, t * 128:(t + 1) * 128],
                    wp_t[:, k * C + lo:k * C + lo + w],
                    start=(k == ks), stop=(k == ke - 1))
            if partial:
                pt = np_.tile([128, 512], f32, tag="pjpart", bufs=8,
                              name=f"pjpart{i}")
                nc.vector.tensor_tensor(
                    out=pt[:, :w], in0=p[:, :w], in1=bp_rep[:, lo:lo + w],
                    op=Add)
                pj_part[(b, i)] = pt
                return
            ot = np_.tile([128, 512], f32, tag="ostage", bufs=3)
            if (b, i) in pj_part:
                nc.vector.tensor_tensor(
                    out=ot[:, :w], in0=p[:, :w], in1=pj_part[(b, i)][:, :w],
                    op=Add)
            else:
                nc.vector.tensor_tensor(
                    out=ot[:, :w], in0=p[:, :w], in1=bp_rep[:, lo:lo + w],
                    op=Add)
            nc.gpsimd.dma_start(
                out_d[t * 128:(t + 1) * 128, lo:lo + w], ot[:, :w])

        # --- software-pipelined emission ---
        for i in range(20):
            qkv_chain(0, i)
        qk1 = iter(range(20))
        for hp in range(6):
            attention_hp(0, hp)
            for _ in range(4 if hp < 2 else 3):
                i = next(qk1, None)
                if i is not None:
                    qkv_chain(1, i)
        for i in qk1:
            qkv_chain(1, i)
        pj0 = iter(range(8))
        for hp in range(6):
            attention_hp(1, hp)
            if hp == 4:
                # heads 0-7 (k<4) of b1 are normalized by now: start their
                # projection partials while hp4/hp5 attention runs
                for i in range(4):
                    proj_chunk(1, i, 0, 4, partial=True)
            if hp == 5:
                for i in range(4, 8):
                    proj_chunk(1, i, 0, 4, partial=True)
            for _ in range(2 if hp >= 2 else 1):
                i = next(pj0, None)
                if i is not None:
                    proj_chunk(0, i)
        for i in pj0:
            proj_chunk(0, i)
        for i in range(8):
            proj_chunk(1, i, 4, KT)

    nc.compile()
    return nc


def get_compiled():
    if "nc" not in _cache:
        _cache["nc"] = _build()
    return _cache["nc"]


def make_in_maps(x, attention_mask, W_attn, b_attn, W_proj, b_proj):
    x = np.asarray(x, dtype=np.float32).astype(np.float16)
    mask = np.ascontiguousarray(
        np.asarray(attention_mask, dtype=np.float32)[:, 0, 0, :])
    wa = np.asarray(W_attn, dtype=np.float32).astype(np.float16)
    ba = np.asarray(b_attn, dtype=np.float32).astype(np.float16).reshape(1, C3)
    wp = np.asarray(W_proj, dtype=np.float32).astype(np.float16)
    bp = np.asarray(b_proj, dtype=np.float32).astype(np.float16).reshape(1, C)
    maps = []
    for i in range(N_CORES):
        maps.append({
            "x": np.ascontiguousarray(x[BC * i:BC * (i + 1)].reshape(M, C)),
            "mask": np.ascontiguousarray(mask[BC * i:BC * (i + 1)]),
            "w_attn": wa, "b_attn": ba, "w_proj": wp, "b_proj": bp,
        })
    return maps


def kernel(x, attention_mask, W_attn, b_attn, W_proj, b_proj):
    from concourse.bass_utils import run_bass_kernel_spmd

    nc = get_compiled()
    in_maps = make_in_maps(x, attention_mask, W_attn, b_attn, W_proj, b_proj)
    last_err = None
    for _ in range(3):
        try:
            res = run_bass_kernel_spmd(nc, in_maps, list(range(N_CORES)))
            break
        except Exception as e:  # transient NRT device errors: retry
            last_err = e
    else:
        raise last_err
    out = np.concatenate(
        [res.results[i]["out"].reshape(BC, T, C) for i in range(N_CORES)], axis=0)
    return out.astype(np.float32)


# revision 10
# speedup vs baseline: 1.1897x; 1.1897x over previous
"""BERT self-attention (B=16, T=512, C=768, H=12, D=64) on 8 trn2 NeuronCores.

Data-parallel over batch: each core gets 2 batches. Matmul operands are fp16
(11-bit mantissa, ~tf32-class precision, 1 cycle/row PE streaming, FWL weight
loads); all accumulation stays fp32 in PSUM. Per core:
  xT    = x transposed during load via the DMA XBAR transpose (fp16).
  Q^T/K^T ([feature, token] layout, lhsT = W_attn tile) and V ([token, feature]
          layout with an interleaved ones column per head, lhsT = xT tile).
  S^T   = K^T-as-lhsT matmul -> scores in [key, query] layout (K=64, head pairs
          packed in PE row groups via base-partition-64 slices).
  P     = exp(S/8 + mask) on ScalarE (mask is a per-partition bias in this
          layout), written as fp16.
  y^T   = lhsT=[V_h | ones] matmul -> unnormalized y^T plus softmax row-sums as
          an extra PSUM row; each head's row-sum is inverted in place with a
          single-pass approx reciprocal, broadcast across 64 partitions by a
          stride-0 DMA, and applied with a DVE multiply fused with the PSUM
          drain.
  out   = y^T-as-lhsT matmul with W_proj + b_proj (fp32 result to DRAM).
Biases are folded in as K=1 matmuls against a ones row / fused into drains.
"""

import sys

sys.path.insert(0, "/opt/trn_rl_repo")

from contextlib import ExitStack

import numpy as np

B, T, C = 16, 512, 768
H, D = 12, 64
C3 = 3 * C
N_CORES = 8
BC = B // N_CORES           # batches per core
M = BC * T                  # tokens per core
KT = C // 128               # feature k-tiles (6)
TT = M // 128               # token tiles per core (8)
NQK = 2 * C // 128          # q+k feature n-tiles (12)
VW = H * 128                # v tile width: per-head [V_h | ones] blocks
SCALE = 1.0 / np.sqrt(D)

_cache = {}


def _build():
    import concourse.bass as bass
    import concourse.tile as tile
    from concourse import bacc, mybir
    f32 = mybir.dt.float32
    f16 = mybir.dt.float16
    Exp = mybir.ActivationFunctionType.Exp
    Mult = mybir.AluOpType.mult
    Add = mybir.AluOpType.add

    nc = bacc.Bacc("TRN2", target_bir_lowering=False, debug=False,
                   num_devices=N_CORES)
    x_d = nc.dram_tensor("x", [M, C], f16, kind="ExternalInput").ap()
    mask_d = nc.dram_tensor("mask", [BC, T], f32, kind="ExternalInput").ap()
    wa_d = nc.dram_tensor("w_attn", [C, C3], f16, kind="ExternalInput").ap()
    ba_d = nc.dram_tensor("b_attn", [1, C3], f16, kind="ExternalInput").ap()
    wp_d = nc.dram_tensor("w_proj", [C, C], f16, kind="ExternalInput").ap()
    bp_d = nc.dram_tensor("b_proj", [1, C], f16, kind="ExternalInput").ap()
    out_d = nc.dram_tensor("out", [M, C], f32, kind="ExternalOutput").ap()

    with tile.TileContext(nc) as tc, ExitStack() as ctx:
        pp = ctx.enter_context(tc.tile_pool(name="pp", bufs=1))
        np_ = ctx.enter_context(tc.tile_pool(name="norm", bufs=4))
        ap_ = ctx.enter_context(tc.tile_pool(name="att", bufs=6))
        ps_mm = ctx.enter_context(tc.tile_pool(name="ps_mm", bufs=2, space="PSUM"))

        # --- persistent SBUF tensors ---
        ones = pp.tile([1, M], f16, tag="ones")
        nc.vector.memset(ones[:], 1.0)
        mask_sb = pp.tile([128, BC * 4], f32, tag="mask")
        ba_t = pp.tile([1, C3], f16, tag="ba")
        bp_t = pp.tile([1, C], f16, tag="bp")
        ba_qk = pp.tile([128, NQK], f32, tag="ba_qk")
        ba_v_rep = pp.tile([128, C], f32, tag="ba_v_rep")
        bp_rep = pp.tile([128, C], f32, tag="bp_rep")

        xT = [pp.tile([128, M], f16, tag=f"xT{k}", name=f"xT{k}")
              for k in range(KT)]
        wa_qk = pp.tile([128, KT * 2 * C], f16, tag="wa_qk")   # [p, (k, n)]
        wa_v = pp.tile([128, KT * C], f16, tag="wa_v")         # [p, (k, n)]
        wp_t = pp.tile([128, KT * C], f16, tag="wp")           # [p, (k, n)]
        qkT = [pp.tile([128, M], f16, tag=f"qk{n}", name=f"qk{n}")
               for n in range(NQK)]
        v_t = [pp.tile([128, VW], f16, tag=f"v{t}", name=f"v{t}")
               for t in range(TT)]
        yT_t = [pp.tile([128, M], f16, tag=f"yT{c}", name=f"yT{c}")
                for c in range(KT)]

        from concourse.masks import make_identity
        ident = pp.tile([128, 128], f16, tag="ident")
        make_identity(nc, ident[:])

        # PE warm-up: harmless K=1 matmuls so the HAM un-throttles before the
        # real chains begin (outputs overwritten by start=True later).
        for w in range(4):
            pw = ps_mm.tile([128, 512], f32, tag="mm", name=f"warm{w}")
            nc.tensor.matmul(pw[:, 0:256], ones[0:1, 0:128],
                             ones[0:1, 0:256], start=True, stop=True)

        # --- input DMAs, consumption order, both HWDGE rings; many
        # medium-size transfers (128 descriptors each) so the rings stream
        # without monopolizing the SDMA engines ---
        xt_ins = []
        for t in range(TT):
            xt_in = pp.tile([128, C], f16, tag=f"x_in{t}", name=f"x_in{t}")
            xt_ins.append(xt_in)
            nc.sync.dma_start(xt_in[:], x_d[t * 128:(t + 1) * 128, :])
        ba12 = pp.tile([NQK, 128], f16, tag="ba12")
        mask8 = pp.tile([BC * 4, 128], f32, tag="mask8")
        nc.scalar.dma_start(ba_t[:], ba_d[:])
        nc.scalar.dma_start(bp_t[:], bp_d[:])
        nc.scalar.dma_start(
            ba12[:], ba_d[0, 0:2 * C].rearrange("(j p) -> j p", p=128))
        nc.scalar.dma_start(
            mask8[:],
            mask_d.rearrange("a b -> (a b)").rearrange("(j p) -> j p", p=128))
        wa_src = wa_d.rearrange("(k p) n -> p k n", p=128)
        wa_qk3 = wa_qk.rearrange("p (k n) -> p k n", n=2 * C)
        for k in range(KT):
            nc.scalar.dma_start(wa_qk3[:, k, :], wa_src[:, k, 0:2 * C])
        for k in range(KT):
            nc.scalar.dma_start(
                wa_v.rearrange("p (k n) -> p k n", n=C)[:, k, :],
                wa_src[:, k, 2 * C:C3])
        for k in range(KT):
            nc.scalar.dma_start(
                wp_t.rearrange("p (k n) -> p k n", n=C)[:, k, :],
                wp_d.rearrange("(k p) n -> p k n", p=128)[:, k, :])

        # ones columns inside the per-head V blocks (col D of each 128 block)
        for t in range(TT):
            nc.vector.memset(
                v_t[t].rearrange("p (h c) -> p h c", c=128)
                    [:, :, D:D + 1], 1.0)

        ps_s = None
        ps_y = None

        with tc.tile_pool(name="ps_tr", bufs=3, space="PSUM") as ps_tr:
            def transpose_tile(t):
                for k in range(KT):
                    ptr = ps_tr.tile([128, 128], f16)
                    nc.tensor.transpose(
                        ptr[:], xt_ins[t][:, k * 128:(k + 1) * 128], ident[:])
                    nc.vector.tensor_copy(
                        xT[k][:, t * 128:(t + 1) * 128], ptr[:])

            for t in range(4):
                transpose_tile(t)
            # small bias/mask tables: row-major DMA + PE transpose (avoids
            # thousand-descriptor rearranged DMAs)
            pb = ps_tr.tile([128, 128], f16, name="ba12T", bufs=1)
            nc.tensor.transpose(pb[0:128, 0:NQK], ba12[:], ident[0:NQK, 0:NQK])
            nc.vector.tensor_copy(ba_qk[:], pb[0:128, 0:NQK])
            ident8 = pp.tile([BC * 4, BC * 4], f32, tag="ident8")
            make_identity(nc, ident8[:])
            pm = ps_tr.tile([128, 128], f32, name="mask8T", bufs=1)
            nc.tensor.transpose(pm[0:128, 0:BC * 4], mask8[:], ident8[:])
            nc.vector.tensor_copy(mask_sb[:], pm[0:128, 0:BC * 4])
            # replicate V/proj biases across partitions (K=1 matmuls)
            for lo, w in ((0, 512), (512, 256)):
                p = ps_mm.tile([128, 512], f32, tag="mm", name=f"brep{lo}")
                nc.tensor.matmul(
                    p[:, :w], ones[0:1, 0:128],
                    ba_t[0:1, 2 * C + lo:2 * C + lo + w], start=True, stop=True)
                nc.vector.tensor_copy(ba_v_rep[:, lo:lo + w], p[:, :w])
                p2 = ps_mm.tile([128, 512], f32, tag="mm", name=f"bprep{lo}")
                nc.tensor.matmul(
                    p2[:, :w], ones[0:1, 0:128],
                    bp_t[0:1, lo:lo + w], start=True, stop=True)
                nc.vector.tensor_copy(bp_rep[:, lo:lo + w], p2[:, :w])
            for t in range(4, TT):
                transpose_tile(t)

        ps_s = ctx.enter_context(tc.tile_pool(name="ps_s", bufs=2, space="PSUM"))
        ps_y = ctx.enter_context(tc.tile_pool(name="ps_y", bufs=2, space="PSUM"))

        Ident = mybir.ActivationFunctionType.Identity

        def qkv_chain(b, i):
            """i in [0, 20): 12 QK n-tiles then 8 V half-tiles."""
            bcol = b * T
            if i < NQK:
                n = i
                p = ps_mm.tile([128, 512], f32, tag="mm", name=f"mm{b}_{i}")
                for k in range(KT):
                    nc.tensor.matmul(
                        p[:],
                        wa_qk[:, k * 2 * C + n * 128:k * 2 * C + (n + 1) * 128],
                        xT[k][:, bcol:bcol + T],
                        start=(k == 0), stop=(k == KT - 1))
                # drain+bias on ScalarE (idle early / has slack) to keep the
                # DVE from gating ps_mm reuse; half stay on DVE for b1.
                if b == 0 or i % 2 == 0:
                    nc.scalar.activation(
                        qkT[n][:, bcol:bcol + T], p[:], Ident,
                        bias=ba_qk[:, n:n + 1])
                else:
                    nc.vector.tensor_scalar_add(
                        qkT[n][:, bcol:bcol + T], p[:], ba_qk[:, n:n + 1])
            else:
                j = i - NQK
                t = b * 4 + j // 2
                lo, w = ((0, 512), (512, 256))[j % 2]
                p = ps_mm.tile([128, 512], f32, tag="mm", name=f"mm{b}_{i}")
                for k in range(KT):
                    nc.tensor.matmul(
                        p[:, :w],
                        xT[k][:, t * 128:(t + 1) * 128],
                        wa_v[:, k * C + lo:k * C + lo + w],
                        start=(k == 0), stop=(k == KT - 1))
                h0 = lo // D
                nc.vector.tensor_tensor(
                    out=v_t[t].rearrange("p (h c) -> p h c", c=128)
                        [:, h0:h0 + w // D, 0:D],
                    in0=p[:, :w].rearrange("p (h c) -> p h c", c=D),
                    in1=ba_v_rep[:, lo:lo + w].rearrange(
                        "p (h c) -> p h c", c=D),
                    op=Add)

        def attention_hp(b, hp):
            bcol = b * T
            e_tiles = []
            for kt in range(4):
                ps = ps_s.tile([128, 1024], f32)
                for sub in range(2):
                    r0 = 64 * sub
                    nc.tensor.matmul(
                        ps[:, sub * 512:sub * 512 + 512],
                        qkT[6 + hp][r0:r0 + D,
                                    bcol + kt * 128:bcol + (kt + 1) * 128],
                        qkT[hp][r0:r0 + D, bcol:bcol + T],
                        start=True, stop=True)
                e = ap_.tile([128, 1024], f16, tag="e")
                nc.scalar.activation(
                    e[:], ps[:], Exp,
                    bias=mask_sb[:, b * 4 + kt:b * 4 + kt + 1],
                    scale=float(SCALE))
                e_tiles.append(e)
            for sub in range(2):
                h = 2 * hp + sub
                py = ps_y.tile([128, 512], f32)
                for kt in range(4):
                    nc.tensor.matmul(
                        py[0:D + 1, :],
                        v_t[b * 4 + kt][:, 128 * h:128 * h + D + 1],
                        e_tiles[kt][:, sub * 512:sub * 512 + 512],
                        start=(kt == 0), stop=(kt == 3))
                # per-head softmax denominators: 1/rowsum straight off PSUM
                recip = np_.tile([D + 1, 512], f32, tag="recip", bufs=3)
                nc.vector.reciprocal_approx_fast(
                    recip[D:D + 1, :], py[D:D + 1, :])
                rep = np_.tile([D, 512], f32, tag="rep", bufs=3)
                nc.gpsimd.partition_broadcast(rep[:], recip[D:D + 1, :])
                nt, r0 = h // 2, 64 * (h % 2)
                dst = yT_t[nt][r0:r0 + D, bcol:bcol + T]
                if r0 == 0:
                    nc.vector.tensor_tensor(
                        out=dst, in0=py[0:D, :], in1=rep[:], op=Mult)
                else:
                    st = np_.tile([D, 512], f16, tag="st", bufs=3)
                    nc.vector.tensor_tensor(
                        out=st[:], in0=py[0:D, :], in1=rep[:], op=Mult)
                    nc.sync.dma_start(dst, st[:])

        pj_part = {}

        def proj_chunk(b, i, ks=0, ke=KT, partial=False):
            t = b * 4 + i // 2
            lo, w = ((0, 512), (512, 256))[i % 2]
            p = ps_mm.tile([128, 512], f32, tag="mm", name=f"pj{b}_{i}_{ks}")
            for k in range(ks, ke):
                nc.tensor.matmul(
                    p[:, :w],
                    yT_t[k][:, t * 128:(t + 1) * 128],
                    wp_t[:, k * C + lo:k * C + lo + w],
                    start=(k == ks), stop=(k == ke - 1))
            if partial:
                pt = np_.tile([128, 512], f32, tag="pjpart", bufs=8,
                              name=f"pjpart{i}")
                nc.vector.tensor_tensor(
                    out=pt[:, :w], in0=p[:, :w], in1=bp_rep[:, lo:lo + w],
                    op=Add)
                pj_part[(b, i)] = pt
                return
            ot = np_.tile([128, 512], f32, tag="ostage", bufs=3)
            if (b, i) in pj_part:
                nc.vector.tensor_tensor(
                    out=ot[:, :w], in0=p[:, :w], in1=pj_part[(b, i)][:, :w],
                    op=Add)
            else:
                nc.vector.tensor_tensor(
                    out=ot[:, :w], in0=p[:, :w], in1=bp_rep[:, lo:lo + w],
                    op=Add)
            nc.gpsimd.dma_start(
                out_d[t * 128:(t + 1) * 128, lo:lo + w], ot[:, :w])

        # --- software-pipelined emission ---
        for i in range(20):
            qkv_chain(0, i)
        qk1 = iter(range(20))
        for hp in range(6):
            attention_hp(0, hp)
            for _ in range(4 if hp < 2 else 3):
                i = next(qk1, None)
                if i is not None:
                    qkv_chain(1, i)
        for i in qk1:
            qkv_chain(1, i)
        pj0 = iter(range(8))
        for hp in range(6):
            attention_hp(1, hp)
            if hp == 4:
                # heads 0-7 (k<4) of b1 are normalized by now: start their
                # projection partials while hp4/hp5 attention runs
                for i in range(4):
                    proj_chunk(1, i, 0, 4, partial=True)
            if hp == 5:
                for i in range(4, 8):
                    proj_chunk(1, i, 0, 4, partial=True)
            for _ in range(2 if hp >= 2 else 1):
                i = next(pj0, None)
                if i is not None:
                    proj_chunk(0, i)
        for i in pj0:
            proj_chunk(0, i)
        for i in range(8):
            proj_chunk(1, i, 4, KT)

    nc.compile()
    return nc


def get_compiled():
    if "nc" not in _cache:
        _cache["nc"] = _build()
    return _cache["nc"]


def make_in_maps(x, attention_mask, W_attn, b_attn, W_proj, b_proj):
    x = np.asarray(x, dtype=np.float32).astype(np.float16)
    mask = np.ascontiguousarray(
        np.asarray(attention_mask, dtype=np.float32)[:, 0, 0, :])
    wa = np.asarray(W_attn, dtype=np.float32).astype(np.float16)
    ba = np.asarray(b_attn, dtype=np.float32).astype(np.float16).reshape(1, C3)
    wp = np.asarray(W_proj, dtype=np.float32).astype(np.float16)
    bp = np.asarray(b_proj, dtype=np.float32).astype(np.float16).reshape(1, C)
    maps = []
    for i in range(N_CORES):
        maps.append({
            "x": np.ascontiguousarray(x[BC * i:BC * (i + 1)].reshape(M, C)),
            "mask": np.ascontiguousarray(mask[BC * i:BC * (i + 1)]),
            "w_attn": wa, "b_attn": ba, "w_proj": wp, "b_proj": bp,
        })
    return maps


def kernel(x, attention_mask, W_attn, b_attn, W_proj, b_proj):
    from concourse.bass_utils import run_bass_kernel_spmd

    nc = get_compiled()
    in_maps = make_in_maps(x, attention_mask, W_attn, b_attn, W_proj, b_proj)
    last_err = None
    for _ in range(3):
        try:
            res = run_bass_kernel_spmd(nc, in_maps, list(range(N_CORES)))
            break
        except Exception as e:  # transient NRT device errors: retry
            last_err = e
    else:
        raise last_err
    out = np.concatenate(
        [res.results[i]["out"].reshape(BC, T, C) for i in range(N_CORES)], axis=0)
    return out.astype(np.float32)
